# revision 22
# baseline (speedup 1.0000x reference)
"""Trainium2 Bass kernel for nn_CausalTrajectoryPrediction.

Per-node stacked MLP over B=16384 rows, N=64 nodes:
  x1[b,i,:] = x[b,:] with entry i zeroed       (mask folded into weights host-side)
  z_i  = relu(W1a'_i @ x) , relu(W2a'_i @ x)   (two branches, packed M=128)
  r_i  = relu(blockdiag(W1b_i, W2b_i) @ z_i)   (K=128, M=64)
  h_i  = relu(W3ab_i @ r_i + w3x_i * x[:,i] + b3a_i)
  out  = relu(w3b_i . h_i + b3b_i)             (final bias+relu on host)

Layout: activations transposed [feature, B]; batch sharded across 8 cores
(BL=2048 each); nodes processed in pairs so every ACT/DVE op uses 128
partitions; matmul groups are subarray-tiled via tile_position for PE
concurrency.  All matmul operands are bf16 (PE runs 4x faster than fp32;
PSUM accumulation stays fp32), and the per-pair L3 mains are merged into
one block-diagonal matmul.  Inputs arrive as 7 prepacked DRAM tensors
(host does all transposes/masking/bf16 casts); built on Bacc so
multi-semaphore waits are split into EventSemaphores.
"""

import numpy as np
from contextlib import ExitStack

N, H, M, B = 64, 64, 32, 16384
NCORES = 8
BL = B // NCORES            # 2048 batch columns per core
CH = 512                    # chunk width (one PSUM bank of fp32)
NPAIR = N // 2              # 32 node pairs

_cache = {}


def _build_bass(bl, npair):
    import concourse.bass as bass
    import concourse.bacc as bacc
    import concourse.mybir as mybir
    import concourse.tile as tile

    F32 = mybir.dt.float32
    BF16 = mybir.dt.bfloat16
    Relu = mybir.ActivationFunctionType.Relu
    Copy = mybir.ActivationFunctionType.Copy
    nch = bl // CH

    nc = bacc.Bacc()
    xt_d = nc.dram_tensor("xt", [128, bl], BF16, kind="ExternalInput")
    w1_d = nc.dram_tensor("w1", [128, npair * 128], BF16, kind="ExternalInput")
    w2_d = nc.dram_tensor("w2", [128, npair * 128], BF16, kind="ExternalInput")
    w3_d = nc.dram_tensor("w3", [128, npair * 128], BF16, kind="ExternalInput")
    w3x_d = nc.dram_tensor("w3x", [N, npair * 128], BF16, kind="ExternalInput")
    w4_d = nc.dram_tensor("w4", [128, npair * 2], BF16, kind="ExternalInput")
    b3a_d = nc.dram_tensor("b3a", [128, npair], F32, kind="ExternalInput")
    out_d = nc.dram_tensor("opre", [bl, N], F32, kind="ExternalOutput")

    with tile.TileContext(nc) as tc, ExitStack() as ctx:
        wpool = ctx.enter_context(tc.tile_pool(name="weights", bufs=1))
        apool = ctx.enter_context(tc.tile_pool(name="acts", bufs=2))
        ps_z = ctx.enter_context(tc.tile_pool(name="ps_z", bufs=3, space="PSUM"))
        ps_r = ctx.enter_context(tc.tile_pool(name="ps_r", bufs=2, space="PSUM"))
        ps_h = ctx.enter_context(tc.tile_pool(name="ps_h", bufs=2, space="PSUM"))
        ps_o = ctx.enter_context(tc.tile_pool(name="ps_o", bufs=1, space="PSUM"))

        # Parallel HWDGE loads, sliced so the first pairs' weights land in
        # ~1us (subtile deps let consumers start per-slice); slice 0 of every
        # tensor is emitted first, then the rest round-robin the queues.
        xt_sb = wpool.tile([128, bl], BF16, tag="xt")
        w1_sb = wpool.tile([128, npair * 128], BF16, tag="w1")
        w2_sb = wpool.tile([128, npair * 128], BF16, tag="w2")
        w3_sb = wpool.tile([128, npair * 128], BF16, tag="w3")
        w3x_sb = wpool.tile([N, npair * 128], BF16, tag="w3x")
        w4_sb = wpool.tile([128, npair * 2], BF16, tag="w4")
        b3a_sb = wpool.tile([128, npair], F32, tag="b3a")
        NS = 4
        wsl = npair * 128 // NS
        xsl = bl // NS
        for s in range(NS):
            nc.sync.dma_start(xt_sb[:, s * xsl:(s + 1) * xsl],
                              xt_d[:, s * xsl:(s + 1) * xsl])
            nc.sync.dma_start(w1_sb[:, s * wsl:(s + 1) * wsl],
                              w1_d[:, s * wsl:(s + 1) * wsl])
            nc.sync.dma_start(w2_sb[:, s * wsl:(s + 1) * wsl],
                              w2_d[:, s * wsl:(s + 1) * wsl])
            nc.sync.dma_start(w3_sb[:, s * wsl:(s + 1) * wsl],
                              w3_d[:, s * wsl:(s + 1) * wsl])
            nc.sync.dma_start(w3x_sb[:, s * wsl:(s + 1) * wsl],
                              w3x_d[:, s * wsl:(s + 1) * wsl])
            if s == 0:
                nc.sync.dma_start(w4_sb[:], w4_d[:])
                nc.sync.dma_start(b3a_sb[:], b3a_d[:])

        iters = [(c, t) for c in range(nch) for t in range(npair)]
        obanks = {}

        def emit_L1(c, t):
            # L1: both branches for each node of the pair (K=64, M=128);
            # the two nodes run row-concurrent on the PE (xt duplicated at
            # partitions 64-127).  Each node gets its own 1-bank PSUM tile
            # and its own relu so downstream L2 can start per-node.
            xt_c = xt_sb[:, c * CH : (c + 1) * CH]
            w1t = w1_sb[:, t * 128 : (t + 1) * 128]
            z_sbs = []
            # DVE carries ~9% more load than ACT; every 16th iteration hand
            # node-1's relu to ACT to even the engines out.
            act_node1 = (c * npair + t) % 16 == 15
            for node in range(2):
                z_ps = ps_z.tile([128, CH], F32, tag="z", name=f"z_{c}_{t}_{node}")
                nc.tensor.matmul(z_ps[:], w1t[64 * node : 64 * node + 64, :],
                                 xt_c[64 * node : 64 * node + 64, :],
                                 start=True, stop=True,
                                 tile_position=(64 * node, 0))
                z_sb = apool.tile([128, CH], BF16, tag="zsb", bufs=6,
                                  name=f"zsb_{c}_{t}_{node}")
                if node == 1 and act_node1:
                    nc.scalar.activation(z_sb[:], z_ps[:], Relu)
                else:
                    nc.vector.tensor_scalar_max(z_sb[:], z_ps[:], 0.0)
                z_sbs.append(z_sb)
            return z_sbs

        def emit_L4(c, t, g_sb):
            # L4 transposed: lhsT=g (M=128 batch cols), rhs=w4 (N=2) ->
            # out [b, node] with nodes on the PSUM free axis
            if c not in obanks:
                obanks[c] = (
                    ps_o.tile([128, 4 * N], F32, tag="o", name=f"o_{c}"),
                    apool.tile([128, 4 * N], F32, tag="osb", name=f"osb_{c}"))
            o_bank, o_sb = obanks[c]
            w4t = w4_sb[:, t * 2 : (t + 1) * 2]
            for bb in range(4):
                nc.tensor.matmul(
                    o_bank[:, bb * N + 2 * t : bb * N + 2 * t + 2],
                    g_sb[:, bb * 128 : (bb + 1) * 128],
                    w4t[:],
                    start=True, stop=True)
            if t == npair - 1:
                nc.scalar.activation(o_sb[:], o_bank[:], Copy)
                nc.sync.dma_start(
                    out_d[c * CH : (c + 1) * CH, :].rearrange(
                        "(bb p) n -> p bb n", p=128),
                    o_sb[:].rearrange("p (bb n) -> p bb n", n=N))

        def emit_L2(c, t, z_sb):
            # L2: block-diag (K=128, M=64) per node, packed into one PSUM
            w2t = w2_sb[:, t * 128 : (t + 1) * 128]
            r_ps = ps_r.tile([128, CH], F32, tag="r")
            nc.tensor.matmul(r_ps[0:64, :], w2t[:, 0:64], z_sb[0][:], start=True, stop=True)
            nc.tensor.matmul(r_ps[64:128, :], w2t[:, 64:128], z_sb[1][:], start=True, stop=True,
                             tile_position=(0, 64))
            f_sb = apool.tile([128, CH], BF16, tag="f", bufs=6)
            nc.scalar.activation(f_sb[:], r_ps[:], Relu)
            return f_sb

        def emit_L3(c, t, f_sb):
            # L3: both nodes' mains as one block-diagonal matmul (zeros in
            # the off-blocks), then the x2 term accumulated on top
            xt_c = xt_sb[:, c * CH : (c + 1) * CH]
            w3t = w3_sb[:, t * 128 : (t + 1) * 128]
            w3xt2 = w3x_sb[:, t * 128 : (t + 1) * 128]
            h_ps = ps_h.tile([128, CH], F32, tag="h")
            nc.tensor.matmul(h_ps[:], w3t[:, :], f_sb[:, :], start=True, stop=False)
            nc.tensor.matmul(h_ps[:], w3xt2[:], xt_c[0:64, :], start=False, stop=True)
            g_sb = apool.tile([128, CH], BF16, tag="g", bufs=6)
            nc.scalar.activation(g_sb[:], h_ps[:], Relu, bias=b3a_sb[:, t : t + 1])
            return g_sb

        # Three-deep software pipeline: per step emit L1(k+2), L2(k+1),
        # L4(k-1), L3(k).  The ~1.3us z-relu (DVE) and ~0.7us f-relu (ACT)
        # latencies are then covered by independent PE work, keeping the PE
        # busy back-to-back.
        niter = len(iters)
        z_sbs = {0: emit_L1(*iters[0])}
        if niter > 1:
            z_sbs[1] = emit_L1(*iters[1])
        f_sbs = {0: emit_L2(*iters[0], z_sbs.pop(0))}
        g_sbs = {}
        for k in range(niter):
            if k + 2 < niter:
                z_sbs[k + 2] = emit_L1(*iters[k + 2])
            if k + 1 < niter:
                f_sbs[k + 1] = emit_L2(*iters[k + 1], z_sbs.pop(k + 1))
            if k >= 1:
                emit_L4(*iters[k - 1], g_sbs.pop(k - 1))
            g_sbs[k] = emit_L3(*iters[k], f_sbs.pop(k))
        emit_L4(*iters[niter - 1], g_sbs.pop(niter - 1))

    nc.compile()
    return nc


def _prep_weights(W1a, W1b, W2a, W2b, W3a, b3a, W3b, npair=NPAIR):
    import ml_dtypes
    BF = np.dtype(ml_dtypes.bfloat16)
    n = W1a.shape[0]
    mask = (1.0 - np.eye(n, dtype=np.float32))  # [i, n]
    W1am = W1a * mask[:, None, :]
    W2am = W2a * mask[:, None, :]
    w1 = np.zeros((npair, 128, 128), np.float32)
    w2 = np.zeros((npair, 128, 128), np.float32)
    w3 = np.zeros((npair, 128, 128), np.float32)
    w3x = np.zeros((npair, n, 128), np.float32)
    w4 = np.zeros((npair, 128, 2), np.float32)
    b3ap = np.zeros((128, npair), np.float32)
    for t in range(npair):
        i0, i1 = 2 * t, 2 * t + 1
        w1[t, 0:64, 0:64] = W1am[i0].T
        w1[t, 0:64, 64:128] = W2am[i0].T
        w1[t, 64:128, 0:64] = W1am[i1].T
        w1[t, 64:128, 64:128] = W2am[i1].T
        w2[t, 0:64, 0:32] = W1b[i0].T
        w2[t, 64:128, 32:64] = W2b[i0].T
        w2[t, 0:64, 64:96] = W1b[i1].T
        w2[t, 64:128, 96:128] = W2b[i1].T
        w3[t, 0:64, 0:64] = W3a[i0][:, 0:64].T
        w3[t, 64:128, 64:128] = W3a[i1][:, 0:64].T
        w3x[t, i0, 0:64] = W3a[i0][:, 64 + i0]
        w3x[t, i1, 64:128] = W3a[i1][:, 64 + i1]
        w4[t, 0:64, 0] = W3b[i0, 0]
        w4[t, 64:128, 1] = W3b[i1, 0]
        b3ap[0:64, t] = b3a[i0]
        b3ap[64:128, t] = b3a[i1]
    # pack pair-major arrays into the SBUF layout [P, npair*F]
    pk = lambda a: np.ascontiguousarray(
        a.transpose(1, 0, 2).reshape(a.shape[1], -1)).astype(BF)
    return pk(w1), pk(w2), pk(w3), pk(w3x), pk(w4), b3ap


def kernel(x, W1a, W1b, W2a, W2b, W3a, b3a, W3b, b3b):
    import ml_dtypes
    from concourse.bass_utils import run_bass_kernel_spmd

    BF = np.dtype(ml_dtypes.bfloat16)
    x = np.asarray(x, np.float32)
    w1, w2, w3, w3x, w4, b3ap = _prep_weights(
        np.asarray(W1a, np.float32), np.asarray(W1b, np.float32),
        np.asarray(W2a, np.float32), np.asarray(W2b, np.float32),
        np.asarray(W3a, np.float32), np.asarray(b3a, np.float32),
        np.asarray(W3b, np.float32))
    b3b = np.asarray(b3b, np.float32)

    if "nc" not in _cache:
        _cache["nc"] = _build_bass(BL, NPAIR)
    nc = _cache["nc"]

    in_maps = []
    for core in range(NCORES):
        xs = x[core * BL : (core + 1) * BL]            # [BL, 64]
        xt = np.ascontiguousarray(
            np.concatenate([xs.T, xs.T], axis=0)).astype(BF)  # [128, BL] dup
        in_maps.append({"xt": xt, "w1": w1, "w2": w2, "w3": w3,
                        "w3x": w3x, "w4": w4, "b3a": b3ap})

    res = run_bass_kernel_spmd(nc, in_maps, core_ids=list(range(NCORES)))
    out = np.empty((B, N), np.float32)
    for core in range(NCORES):
        opre = res.results[core]["opre"]               # [BL, 64]
        out[core * BL : (core + 1) * BL] = np.maximum(opre + b3b[:, 0][None, :], 0.0)
    return out
